# revision 2
# baseline (speedup 1.0000x reference)
"""Trainium2 Bass kernel v2 for nn_DepthAwareGATv2 (2-layer GATv2, 8 cores).

Design (vs baseline): full Python unroll (no hw loops/drains), dma_gather
row-gathers (994ns trigger amortized over a whole tile), global-index node
tables computed fully per core (no ag1), edge-side a_src computed from
gathered xs, dst-side x/adst fetched by dst-keyed gathers from local
tables, single activation table (exp/ln/relu), LN rstd = exp(-0.5*ln(v)).

Per core: nodes [k*6250,(k+1)*6250), edges sharded by dst owner, sorted by
dst, tiled 124 dst-slots/tile, edge slots chunked 128/partition-dim. Edge
order within a tile: src-row group0 (<GS) then group1 (>=GS), each padded
to a 128 multiple; pad edges use src 0 / dst 0 / slot 127.
"""

import math
import os

import numpy as np

NCORES = 8
LAST_EXEC_NS = None
LAST_SCOPES = None
P = 128
NEG_SLOPE = 0.2
NPB = 124
GS = 32768          # int16 gather group split (table row space)

_CACHE = {}


def _bf16(a):
    import ml_dtypes
    return np.ascontiguousarray(
        np.asarray(a, dtype=np.float32).astype(ml_dtypes.bfloat16))


def _pack_idx16(idx, ch):
    """[n] int array -> dma_gather layout [128, ch*8] i16 (i at [i%16,i//16])."""
    n = idx.shape[0]
    assert n == ch * 128
    stripe = idx.astype(np.int16).reshape(ch * 8, 16).T
    return np.ascontiguousarray(np.tile(stripe, (8, 1)))


def _preprocess(edge_index, n, nsh):
    src_all = edge_index[0].astype(np.int64)
    dst_all = edge_index[1].astype(np.int64)
    t_cnt = (nsh + NPB - 1) // NPB
    percore = []
    c0max = c1max = 0
    for k in range(NCORES):
        n0 = k * nsh
        m = (dst_all // nsh) == k
        src, dst = src_all[m], dst_all[m] - n0
        order = np.argsort(dst, kind="stable")
        src, dst = src[order], dst[order]
        tiles = []
        for t in range(t_cnt):
            lo = np.searchsorted(dst, t * NPB)
            hi = np.searchsorted(dst, (t + 1) * NPB)
            s, d = src[lo:hi], dst[lo:hi]
            g0 = s < GS
            s0, d0 = s[g0], d[g0]
            s1, d1 = s[~g0], d[~g0]
            c0 = (len(s0) + P - 1) // P
            c1 = (len(s1) + P - 1) // P
            c0max, c1max = max(c0max, c0), max(c1max, c1)
            tiles.append((s0, d0, s1, d1))
        percore.append(tiles)
    ch0, ch1 = c0max, c1max
    ch = ch0 + ch1
    out = []
    for k, tiles in enumerate(percore):
        src_i16 = np.zeros((t_cnt, P, ch * 8), np.int16)
        dst_i16 = np.zeros((t_cnt, P, ch * 8), np.int16)
        slot = np.full((t_cnt, ch * P), 127.0, np.float32)
        for t, (s0, d0, s1, d1) in enumerate(tiles):
            sg = np.zeros(ch * P, np.int64)
            dg = np.zeros(ch * P, np.int64)
            sl = np.full(ch * P, 127, np.int64)
            sg[:len(s0)] = s0
            dg[:len(d0)] = d0
            sl[:len(s0)] = d0 - t * NPB
            o1 = ch0 * P
            sg[o1:o1 + len(s1)] = s1 - GS
            dg[o1:o1 + len(s1)] = d1
            sl[o1:o1 + len(s1)] = d1 - t * NPB
            src_i16[t] = _pack_idx16(sg, ch)
            dst_i16[t] = _pack_idx16(dg, ch)
            slot[t] = sl.astype(np.float32)
        # slot in edge-major [p, t*ch+c] layout: edge i=c*128+p
        slot_pc = slot.reshape(t_cnt, ch, P).transpose(2, 0, 1).reshape(
            P, t_cnt * ch)
        out.append((src_i16, dst_i16, np.ascontiguousarray(slot_pc), slot))
    return ch0, ch1, out


def _fold_weights(I):
    H, HID = I["c1_att_src"].shape
    F = I["x"].shape[1]

    def v(lin_w, att):
        return (lin_w.reshape(lin_w.shape[0], H, HID) * att[None]).sum(-1)

    W = {}
    G1 = np.einsum("jhc,hc->jh", I["c1_edge_w"].reshape(H, H, HID),
                   I["c1_att_edge"])
    G2 = np.einsum("jhc,hc->jh", I["c2_edge_w"].reshape(H, H, HID),
                   I["c2_att_edge"])
    mboth = np.concatenate([I["ee_w2"] @ G1, I["ee_w2"] @ G2], axis=1)  # [16,16]
    W["mboth_bd"] = np.kron(np.eye(8, dtype=np.float32), mboth)        # [128,128]
    w1e = I["ee_w1"] * (1.0 / F) ** np.arange(1, 5)[:, None]           # [4,16]
    W["w1e_bd"] = np.kron(np.eye(8, dtype=np.float32), w1e)            # [32,128]
    W["eeb1_rep"] = np.tile(I["ee_b1"], 8).reshape(P, 1).astype(np.float32)
    dd = np.concatenate([I["ee_b2"] @ G1, I["ee_b2"] @ G2])            # [16]
    W["dd"] = dd
    eye = np.eye(F, dtype=np.float32)
    W["wn1g"] = np.concatenate(
        [eye, I["c1_lin_w"], v(I["c1_lin_w"], I["c1_att_src"])], axis=1)  # [128,264]
    W["wn1l"] = np.concatenate(
        [eye, v(I["c1_lin_w"], I["c1_att_dst"]), I["in_w"]], axis=1)   # [128,264]
    W["wn2"] = np.concatenate(
        [I["c2_lin_w"], v(I["c2_lin_w"], I["c2_att_src"]),
         v(I["c2_lin_w"], I["c2_att_dst"])], axis=1)                   # [128,144]
    W["asrc1"] = v(I["c1_lin_w"], I["c1_att_src"])                     # [128,8] unused
    # att_src replicated for on-edge a_src: [128, 8,16] rows identical
    W["att1_rep"] = np.tile(I["c1_att_src"].reshape(1, H * HID), (P, 1))
    W["att2_rep"] = np.tile(I["c2_att_src"].reshape(1, H * HID), (P, 1))
    W["jkw"] = np.stack([I["jk_w"][0:128], I["jk_w"][128:256],
                         I["jk_w"][256:384]], axis=1)                  # [128,3,128]
    W["clsw"] = I["cls_w"]
    clsb = I["cls_b"] + I["jk_b"] @ I["cls_w"]
    W["clsb"] = np.tile(clsb[None, :], (P, 1)).astype(np.float32)
    return W


def _build(cfg):
    import concourse.bass as bass
    import concourse.mybir as mybir
    from concourse.bacc import Bacc
    from concourse.tile import TileContext
    from concourse import library_config
    from concourse.masks import make_identity

    f32 = mybir.dt.float32
    bf = mybir.dt.bfloat16
    i16 = mybir.dt.int16
    AF = mybir.ActivationFunctionType
    OP = mybir.AluOpType
    AX = mybir.AxisListType

    NSH = cfg["NSH"]              # 6250
    T = cfg["T"]                  # 51
    CH0, CH1 = cfg["CH0"], cfg["CH1"]
    CH = CH0 + CH1
    NCHK = (NSH + P - 1) // P     # 49 (last chunk 106)
    NPAD = cfg["NPAD"]            # 50048
    JCH = NPAD // P               # 391
    NLOC = T * NPB                # 6324
    NLOCP = ((NLOC + P - 1) // P) * P  # 6400
    ncores = cfg["ncores"]

    nc = Bacc(num_devices=ncores, dynamic_dma_scratch_size=16384,
              num_swdge_queues=4)

    xtf = nc.dram_tensor("xtf", [P, NPAD], bf, kind="ExternalInput")
    xtl = nc.dram_tensor("xtl", [P, NLOCP], bf, kind="ExternalInput")
    idx_src = nc.dram_tensor("idx_src", [T, P, CH * 8], i16, kind="ExternalInput")
    slot_pc = nc.dram_tensor("slot_pc", [P, T * CH], bf, kind="ExternalInput")
    slot_row = nc.dram_tensor("slot_row", [T, CH * P], bf, kind="ExternalInput")
    iota_c = nc.dram_tensor("iota_c", [P, P], bf, kind="ExternalInput")
    iotap_c = nc.dram_tensor("iotap_c", [P, 1], f32, kind="ExternalInput")
    ones_c = nc.dram_tensor("ones_c", [1, NPB], bf, kind="ExternalInput")
    wn1g = nc.dram_tensor("wn1g", [P, 264], bf, kind="ExternalInput")
    wn1l = nc.dram_tensor("wn1l", [P, 264], bf, kind="ExternalInput")
    wn2 = nc.dram_tensor("wn2", [P, 144], bf, kind="ExternalInput")
    w1e_bd = nc.dram_tensor("w1e_bd", [32, P], bf, kind="ExternalInput")
    mboth_bd = nc.dram_tensor("mboth_bd", [P, P], bf, kind="ExternalInput")
    eeb1_rep = nc.dram_tensor("eeb1_rep", [P, 1], f32, kind="ExternalInput")
    att1_rep = nc.dram_tensor("att1_rep", [P, P], bf, kind="ExternalInput")
    att2_rep = nc.dram_tensor("att2_rep", [P, P], bf, kind="ExternalInput")
    jkw = nc.dram_tensor("jkw", [P, 3, P], bf, kind="ExternalInput")
    clsw = nc.dram_tensor("clsw", [P, 40], bf, kind="ExternalInput")
    clsb = nc.dram_tensor("clsb", [P, 40], f32, kind="ExternalInput")

    out_t = nc.dram_tensor("out", [NSH, 40], f32, kind="ExternalOutput")

    with TileContext(nc) as tc:
        with (
            tc.tile_pool(name="dram", bufs=1, space="DRAM") as dpool,
            tc.tile_pool(name="consts", bufs=1) as cpool,
            tc.tile_pool(name="persist", bufs=1) as ppool,
            tc.tile_pool(name="p1pool", bufs=4) as p1pool,
            tc.tile_pool(name="gpool", bufs=2) as gpool,
            tc.tile_pool(name="epool", bufs=2) as epool,
            tc.tile_pool(name="npool", bufs=2) as npool,
            tc.tile_pool(name="pshare", bufs=6, space="PSUM") as pshare,
            tc.tile_pool(name="pagg", bufs=2, space="PSUM") as pagg,
        ):
            shared = {"addr_space": "Shared"} if ncores > 1 else {}
            t1ex = dpool.tile([NPAD, 384], bf, name="t1ex")
            t1loc = dpool.tile([NLOCP, 136], bf, name="t1loc")
            t2loc = dpool.tile([NSH, 256], bf, name="t2loc")
            t2full = dpool.tile([NSH * ncores, 256], bf, name="t2full", **shared)
            dst2ex = dpool.tile([NLOCP, 8], bf, name="dst2ex")
            agg1 = dpool.tile([NLOCP, 136], f32, name="agg1")
            agg2 = dpool.tile([NLOCP, 136], f32, name="agg2")

            def ld(shape, dt_, src, pool=cpool):
                t = pool.tile(shape, dt_, name=f"c_{src.name}")
                nc.sync.dma_start(out=t[:], in_=src[:])
                return t

            iota_t = ld([P, P], bf, iota_c)
            iotap_t = ld([P, 1], f32, iotap_c)
            ones_t = ld([1, NPB], bf, ones_c)
            slot_t = ld([P, T * CH], bf, slot_pc)
            wn1g_t = ld([P, 264], bf, wn1g)
            wn1l_t = ld([P, 264], bf, wn1l)
            wn2_t = ld([P, 144], bf, wn2)
            w1e_t = ld([32, P], bf, w1e_bd)
            mb_t = ld([P, P], bf, mboth_bd)
            eeb1_t = ld([P, 1], f32, eeb1_rep)
            jkw_t = ld([P, 3, P], bf, jkw)
            clsw_t = ld([P, 40], bf, clsw)
            clsb_t = ld([P, 40], f32, clsb)
            h0T = ppool.tile([P, NLOCP], bf, name="h0T")
            h1T = ppool.tile([P, NLOCP], bf, name="h1T")
            ae_sb = ppool.tile([P, T, CH, 8], bf, name="ae_sb")
            ident = cpool.tile([P, P], bf, name="ident")
            make_identity(nc, ident[:])
            eps_t = cpool.tile([P, 1], f32, name="eps_t")
            nc.vector.memset(eps_t[:], 1e-5)
            nc.gpsimd.load_library(library_config.mlp)

            # ============ p1: full t1ex table + local t1loc/h0 ============
            with nc.named_scope("p1"):
                zrow = cpool.tile([P, 136], bf, name="zrow")
                nc.vector.memset(zrow[:], 0.0)
                for r0 in range(NSH, NLOCP, P):
                    rw = min(P, NLOCP - r0)
                    nc.sync.dma_start(out=t1loc[bass.ds(r0, rw), :],
                                      in_=zrow[0:rw, :])
                    nc.sync.dma_start(out=dst2ex[bass.ds(r0, rw), :],
                                      in_=zrow[0:rw, 0:8])
                for j in range(JCH):
                    xc = p1pool.tile([P, P], bf, tag="xc")
                    nc.sync.dma_start(out=xc[:], in_=xtf[:, bass.ds(j * P, P)])
                    ps = pshare.tile([P, 264], f32, tag="ps")
                    nc.tensor.matmul(out=ps[:, 0:264], lhsT=xc[:], rhs=wn1g_t[:],
                                     start=True, stop=True)
                    row = p1pool.tile([P, 264], bf, tag="p1row")
                    nc.scalar.activation(out=row[:], in_=ps[:, 0:264],
                                         func=AF.Copy)
                    nc.sync.dma_start(out=t1ex[bass.ds(j * P, P), 0:264],
                                      in_=row[:])
                for j in range(NCHK):
                    nr = min(P, NSH - j * P)
                    xc = p1pool.tile([P, P], bf, tag="xlc")
                    nc.sync.dma_start(out=xc[:, 0:nr],
                                      in_=xtl[:, bass.ds(j * P, nr)])
                    ps = pshare.tile([P, 264], f32, tag="ps")
                    nc.tensor.matmul(out=ps[0:nr, :], lhsT=xc[:, 0:nr],
                                     rhs=wn1l_t[:], start=True, stop=True)
                    row = p1pool.tile([P, 136], bf, tag="p1lrow")
                    nc.vector.tensor_copy(out=row[0:nr, :], in_=ps[0:nr, 0:136])
                    nc.sync.dma_start(out=t1loc[bass.ds(j * P, nr), 0:136],
                                      in_=row[0:nr, :])
                    h0b = p1pool.tile([P, P], bf, tag="h0b")
                    nc.vector.tensor_copy(out=h0b[0:nr, :], in_=ps[0:nr, 136:264])
                    h0tp = pshare.tile([P, P], bf, tag="ps")
                    nc.tensor.transpose(out=h0tp[:, 0:nr], in_=h0b[0:nr, :],
                                        identity=ident[0:nr, 0:nr])
                    nc.vector.tensor_copy(out=h0T[:, bass.ds(j * P, nr)],
                                          in_=h0tp[:, 0:nr])

            # ============ edge passes ============
            qn = [0]

            def layer_pass(scope, layer, tblg, tbld, agg_dst):
                xs_off = 128 if layer == 1 else 0
                as_off = 256 if layer == 1 else 128
                gcols = 384 if layer == 1 else 256
                dcols = 136 if layer == 1 else 8
                with nc.named_scope(scope):
                    for t in range(T):
                        isrc = gpool.tile([P, CH * 8], i16, tag="isrc", bufs=3)
                        nc.sync.dma_start(out=isrc[:], in_=idx_src[t])
                        g = gpool.tile([P, CH, gcols], bf, tag="g", bufs=2)
                        for g0, g1, base in ((0, CH0, 0), (CH0, CH, GS)):
                            c0 = g0
                            while c0 < g1:
                                w = min(7, g1 - c0)
                                src_ap = (tblg[:] if base == 0 else
                                          tblg[bass.ds(GS, tblg.shape[0] - GS), :])
                                nc.gpsimd.dma_gather(
                                    g[:, c0:c0 + w, :], src_ap,
                                    isrc[:, c0 * 8:(c0 + w) * 8],
                                    w * P, w * P, gcols,
                                    queue_num=qn[0] % 4)
                                qn[0] += 1
                                c0 += w
                        # dst-side: 124 rows -> ohT -> per-chunk broadcast
                        dslots = gpool.tile([NPB, dcols], bf, tag="dsl")
                        nc.sync.dma_start(out=dslots[:],
                                          in_=tbld[bass.ds(t * NPB, NPB), :])
                        srow = gpool.tile([1, CH * P], bf, tag="srow",
                                          bufs=2)
                        nc.sync.dma_start(out=srow[:],
                                          in_=slot_row[bass.ds(t, 1), :])
                        ohT = gpool.tile([NPB, CH * P], bf, tag="ohT")
                        for q0 in range(0, CH * P, 512):
                            qw = min(512, CH * P - q0)
                            sb = pshare.tile([NPB, 512], f32, tag="ps")
                            nc.tensor.matmul(
                                out=sb[:, 0:qw], lhsT=ones_t[:],
                                rhs=srow[:, bass.ds(q0, qw)],
                                start=True, stop=True)
                            nc.vector.tensor_scalar(
                                out=ohT[:, q0:q0 + qw], in0=sb[:, 0:qw],
                                scalar1=iotap_t[0:NPB, :], scalar2=None,
                                op0=OP.is_equal)
                        xda = gpool.tile([P, CH, dcols], bf, tag="xda")
                        for c in range(CH):
                            xd_ps = pshare.tile([P, dcols], f32, tag="ps")
                            nc.tensor.matmul(
                                out=xd_ps[:], lhsT=ohT[:, bass.ds(c * P, P)],
                                rhs=dslots[:], start=True, stop=True)
                            nc.scalar.activation(out=xda[:, c, :],
                                                 in_=xd_ps[:], func=AF.Copy)
                        slot_v = slot_t[:, bass.ds(t * CH, CH)]
                        # z = a_src(table) + a_dst(bcast) + ae
                        z = gpool.tile([P, CH, 8], f32, tag="z")
                        nc.vector.tensor_tensor(
                            out=z[:], in0=g[:, :, as_off:as_off + 8],
                            in1=xda[:, :, 128:136] if layer == 1 else xda[:],
                            op=OP.add)
                        if layer == 1:
                            # s-dot + edge MLP -> ae (both layers)
                            prod = gpool.tile([P, CH, P], bf, tag="prod")
                            nc.vector.tensor_tensor(out=prod[:],
                                                    in0=g[:, :, 0:128],
                                                    in1=xda[:, :, 0:128],
                                                    op=OP.mult)
                            s = gpool.tile([P, CH], f32, tag="s")
                            nc.vector.tensor_reduce(out=s[:], in_=prod[:],
                                                    op=OP.add, axis=AX.X)
                            s2 = gpool.tile([P, CH], f32, tag="s2")
                            nc.vector.tensor_tensor(out=s2[:], in0=s[:],
                                                    in1=s[:], op=OP.mult)
                            p4 = gpool.tile([P, CH, 4], bf, tag="p4")
                            nc.vector.tensor_copy(out=p4[:, :, 0], in_=s[:])
                            nc.vector.tensor_copy(out=p4[:, :, 1], in_=s2[:])
                            s34 = gpool.tile([P, CH], f32, tag="s34")
                            nc.vector.tensor_tensor(out=s34[:], in0=s2[:],
                                                    in1=s[:], op=OP.mult)
                            nc.vector.tensor_copy(out=p4[:, :, 2], in_=s34[:])
                            nc.vector.tensor_tensor(out=s34[:], in0=s2[:],
                                                    in1=s2[:], op=OP.mult)
                            nc.vector.tensor_copy(out=p4[:, :, 3], in_=s34[:])
                            ae_t = gpool.tile([P, CH, 16], bf, tag="ae_t")
                            for gi in range(0, CH, 8):
                                w = min(8, CH - gi)
                                ptp = pshare.tile([32, P], bf, tag="ps")
                                nc.tensor.transpose(
                                    out=ptp[0:4 * w, :],
                                    in_=p4[:, gi:gi + w, :].rearrange(
                                        "p c f -> p (c f)"),
                                    identity=ident[:])
                                p4T = gpool.tile([32, P], bf, tag="p4T")
                                nc.scalar.activation(out=p4T[0:4 * w, :],
                                                     in_=ptp[0:4 * w, :],
                                                     func=AF.Copy)
                                hid = pshare.tile([P, P], f32, tag="ps")
                                nc.tensor.matmul(out=hid[0:16 * w, :],
                                                 lhsT=w1e_t[0:4 * w, 0:16 * w],
                                                 rhs=p4T[0:4 * w, :],
                                                 start=True, stop=True)
                                hidr = gpool.tile([P, P], bf, tag="hidr")
                                nc.scalar.activation(
                                    out=hidr[0:16 * w, :], in_=hid[0:16 * w, :],
                                    func=AF.Relu, bias=eeb1_t[0:16 * w, :])
                                aeps = pshare.tile([P, P], f32, tag="ps")
                                nc.tensor.matmul(out=aeps[:, 0:16 * w],
                                                 lhsT=hidr[0:16 * w, :],
                                                 rhs=mb_t[0:16 * w, 0:16 * w],
                                                 start=True, stop=True)
                                nc.scalar.activation(
                                    out=ae_t[:, gi:gi + w, :],
                                    in_=aeps[:, 0:16 * w].rearrange(
                                        "p (c f) -> p c f", f=16),
                                    func=AF.Copy)
                            nc.vector.tensor_copy(out=ae_sb[:, t, :, :],
                                                  in_=ae_t[:, :, 8:16])
                            ae_l = ae_t[:, :, 0:8]
                        else:
                            ae_l = ae_sb[:, t, :, :]
                        nc.vector.tensor_tensor(out=z[:], in0=z[:], in1=ae_l,
                                                op=OP.add)
                        if cfg["has_dd"]:
                            raise NotImplementedError
                        zz = gpool.tile([P, CH, 8], f32, tag="zz")
                        nc.vector.tensor_scalar(out=zz[:], in0=z[:],
                                                scalar1=NEG_SLOPE, scalar2=None,
                                                op0=OP.mult)
                        nc.vector.tensor_tensor(out=z[:], in0=z[:], in1=zz[:],
                                                op=OP.max)
                        mez = gpool.tile([P, CH, 136], bf, tag="mez")
                        nc.scalar.activation(out=mez[:, :, 128:136], in_=z[:],
                                             func=AF.Exp)
                        nc.vector.tensor_tensor(
                            out=mez[:, :, 0:128].rearrange(
                                "p c (h q) -> p c h q", h=8),
                            in0=g[:, :, xs_off:xs_off + 128].rearrange(
                                "p c (h q) -> p c h q", h=8),
                            in1=mez[:, :, 128:136, None].to_broadcast(
                                [P, CH, 8, 16]),
                            op=OP.mult)
                        oh = gpool.tile([P, CH, NPB], bf, tag="oh")
                        nc.vector.tensor_tensor(
                            out=oh[:],
                            in0=iota_t[:, None, 0:NPB].to_broadcast(
                                [P, CH, NPB]),
                            in1=slot_v[:, :, None].to_broadcast([P, CH, NPB]),
                            op=OP.is_equal)
                        aggp = pagg.tile([NPB, 136], f32, tag="aggp")
                        for c in range(CH):
                            nc.tensor.matmul(out=aggp[:], lhsT=oh[:, c, :],
                                             rhs=mez[:, c, :],
                                             start=(c == 0), stop=(c == CH - 1))
                        aggs = gpool.tile([NPB, 136], f32, tag="aggs")
                        nc.vector.tensor_copy(out=aggs[:], in_=aggp[:])
                        nc.sync.dma_start(out=agg_dst[bass.ds(t * NPB, NPB), :],
                                          in_=aggs[:])

            layer_pass("passA", 1, t1ex, t1loc, agg1)

            # ============ epilogues ============
            def epilogue(scope, agg_src, hprevT, build_t2):
                with nc.named_scope(scope):
                    for j in range(NCHK):
                        nr = min(P, NSH - j * P)
                        ag = epool.tile([P, 136], f32, tag="ag")
                        nc.sync.dma_start(out=ag[0:nr, :],
                                          in_=agg_src[bass.ds(j * P, nr), :])
                        rden = epool.tile([P, 8], f32, tag="rden")
                        nc.vector.reciprocal(out=rden[0:nr, :],
                                             in_=ag[0:nr, 128:136])
                        o1 = epool.tile([P, P], f32, tag="o1")
                        nc.vector.tensor_tensor(
                            out=o1[0:nr, :].rearrange("p (h q) -> p h q", h=8),
                            in0=ag[0:nr, 0:128].rearrange(
                                "p (h q) -> p h q", h=8),
                            in1=rden[0:nr, :, None].to_broadcast([nr, 8, 16]),
                            op=OP.mult)
                        # elu + residual(hprev)
                        mn = epool.tile([P, P], f32, tag="mn")
                        nc.vector.tensor_scalar(out=mn[0:nr, :], in0=o1[0:nr, :],
                                                scalar1=0.0, scalar2=None,
                                                op0=OP.min)
                        ex = epool.tile([P, P], f32, tag="ex")
                        nc.scalar.activation(out=ex[0:nr, :], in_=mn[0:nr, :],
                                             func=AF.Exp)
                        h = epool.tile([P, P], f32, tag="h")
                        nc.vector.tensor_scalar(out=h[0:nr, :], in0=o1[0:nr, :],
                                                scalar1=0.0, scalar2=None,
                                                op0=OP.max)
                        nc.vector.tensor_tensor(out=h[0:nr, :], in0=h[0:nr, :],
                                                in1=ex[0:nr, :], op=OP.add)
                        hptp = pshare.tile([P, P], bf, tag="ps")
                        nc.tensor.transpose(out=hptp[0:nr, :],
                                            in_=hprevT[:, bass.ds(j * P, nr)],
                                            identity=ident[:])
                        nc.vector.tensor_tensor(out=h[0:nr, :], in0=h[0:nr, :],
                                                in1=hptp[0:nr, :], op=OP.add)
                        nc.vector.tensor_scalar(out=h[0:nr, :], in0=h[0:nr, :],
                                                scalar1=-1.0, scalar2=None,
                                                op0=OP.add)
                        # layernorm: rstd = exp(-0.5*ln(var+eps))
                        msum = epool.tile([P, 1], f32, tag="msum")
                        nc.vector.tensor_reduce(out=msum[0:nr, :],
                                                in_=h[0:nr, :], op=OP.add,
                                                axis=AX.X)
                        nc.vector.tensor_scalar(out=msum[0:nr, :],
                                                in0=msum[0:nr, :],
                                                scalar1=1.0 / 128, scalar2=None,
                                                op0=OP.mult)
                        xc = epool.tile([P, P], f32, tag="xc")
                        nc.vector.tensor_scalar(out=xc[0:nr, :], in0=h[0:nr, :],
                                                scalar1=msum[0:nr, :],
                                                scalar2=None, op0=OP.subtract)
                        sq = epool.tile([P, P], f32, tag="sq")
                        nc.vector.tensor_tensor(out=sq[0:nr, :], in0=xc[0:nr, :],
                                                in1=xc[0:nr, :], op=OP.mult)
                        vs = epool.tile([P, 1], f32, tag="vs")
                        nc.vector.tensor_reduce(out=vs[0:nr, :], in_=sq[0:nr, :],
                                                op=OP.add, axis=AX.X)
                        lnv = epool.tile([P, 1], f32, tag="lnv")
                        nc.scalar.activation(out=lnv[0:nr, :], in_=vs[0:nr, :],
                                             func=AF.Ln, scale=1.0 / 128,
                                             bias=eps_t[0:nr, :])
                        rstd = epool.tile([P, 1], f32, tag="rstd")
                        nc.scalar.activation(out=rstd[0:nr, :], in_=lnv[0:nr, :],
                                             func=AF.Exp, scale=-0.5)
                        hln = epool.tile([P, P], f32, tag="hln")
                        nc.vector.tensor_scalar(out=hln[0:nr, :],
                                                in0=xc[0:nr, :],
                                                scalar1=rstd[0:nr, :],
                                                scalar2=None, op0=OP.mult)
                        hb = epool.tile([P, P], bf, tag="hb")
                        nc.vector.tensor_copy(out=hb[0:nr, :], in_=hln[0:nr, :])
                        htp = pshare.tile([P, P], bf, tag="ps")
                        nc.tensor.transpose(out=htp[:, 0:nr], in_=hb[0:nr, :],
                                            identity=ident[0:nr, 0:nr])
                        if build_t2:
                            nc.vector.tensor_copy(
                                out=h1T[:, bass.ds(j * P, nr)],
                                in_=htp[:, 0:nr])
                            t2ps = pshare.tile([P, 136], f32, tag="ps")
                            nc.tensor.matmul(out=t2ps[0:nr, :],
                                             lhsT=h1T[:, bass.ds(j * P, nr)],
                                             rhs=wn2_t[:], start=True, stop=True)
                            t2row = epool.tile([P, P], bf, tag="t2row")
                            nc.vector.tensor_copy(out=t2row[0:nr, :],
                                                  in_=t2ps[0:nr, 0:128])
                            nc.sync.dma_start(out=t2loc[bass.ds(j * P, nr), :],
                                              in_=t2row[0:nr, :])
                            adrow = epool.tile([P, 8], bf, tag="adrow")
                            nc.vector.tensor_copy(out=adrow[0:nr, :],
                                                  in_=t2ps[0:nr, 128:136])
                            nc.sync.dma_start(
                                out=dst2ex[bass.ds(j * P, nr), :],
                                in_=adrow[0:nr, :])
                        else:
                            h2T = npool.tile([P, P], bf, tag="h2T")
                            nc.vector.tensor_copy(out=h2T[:, 0:nr],
                                                  in_=htp[:, 0:nr])
                            hh = pshare.tile([P, P], f32, tag="ps")
                            nc.tensor.matmul(out=hh[:, 0:nr], lhsT=jkw_t[:, 0, :],
                                             rhs=h0T[:, bass.ds(j * P, nr)],
                                             start=True, stop=False)
                            nc.tensor.matmul(out=hh[:, 0:nr], lhsT=jkw_t[:, 1, :],
                                             rhs=h1T[:, bass.ds(j * P, nr)],
                                             start=False, stop=False)
                            nc.tensor.matmul(out=hh[:, 0:nr], lhsT=jkw_t[:, 2, :],
                                             rhs=h2T[:, 0:nr],
                                             start=False, stop=True)
                            hhb = npool.tile([P, P], bf, tag="hhb")
                            nc.vector.tensor_copy(out=hhb[:, 0:nr],
                                                  in_=hh[:, 0:nr])
                            lg = pshare.tile([P, 40], f32, tag="ps")
                            nc.tensor.matmul(out=lg[0:nr, :],
                                             lhsT=hhb[:, 0:nr], rhs=clsw_t[:],
                                             start=True, stop=True)
                            if cfg["has_clsb"]:
                                raise NotImplementedError
                            rmax = npool.tile([P, 1], f32, tag="rmax")
                            nc.vector.tensor_reduce(out=rmax[0:nr, :],
                                                    in_=lg[0:nr, :], op=OP.max,
                                                    axis=AX.X)
                            xm = npool.tile([P, 40], f32, tag="xm")
                            nc.vector.tensor_scalar(out=xm[0:nr, :],
                                                    in0=lg[0:nr, :],
                                                    scalar1=rmax[0:nr, :],
                                                    scalar2=None,
                                                    op0=OP.subtract)
                            ee = npool.tile([P, 40], f32, tag="ee")
                            esum = npool.tile([P, 1], f32, tag="esum")
                            nc.scalar.activation(out=ee[0:nr, :], in_=xm[0:nr, :],
                                                 func=AF.Exp,
                                                 accum_out=esum[0:nr, :])
                            lse = npool.tile([P, 1], f32, tag="lse")
                            nc.scalar.activation(out=lse[0:nr, :],
                                                 in_=esum[0:nr, :], func=AF.Ln)
                            res = npool.tile([P, 40], f32, tag="res")
                            nc.vector.tensor_scalar(out=res[0:nr, :],
                                                    in0=xm[0:nr, :],
                                                    scalar1=lse[0:nr, :],
                                                    scalar2=None,
                                                    op0=OP.subtract)
                            nc.sync.dma_start(out=out_t[bass.ds(j * P, nr), :],
                                              in_=res[0:nr, :])

            epilogue("ep1", agg1, h0T, True)

            with nc.named_scope("ag2"):
                if ncores > 1:
                    import concourse.mybir as mybir2
                    nc.gpsimd.collective_compute(
                        "AllGather", mybir2.AluOpType.bypass,
                        replica_groups=[list(range(ncores))],
                        ins=[t2loc[:]], outs=[t2full[:]],
                    )
                else:
                    nc.sync.dma_start(out=t2full[:], in_=t2loc[:])

            layer_pass("passB", 2, t2full, dst2ex, agg2)
            epilogue("ep2", agg2, h1T, False)

    nc.finalize()
    return nc


def _prepare(inputs):
    I = {k: np.asarray(v) for k, v in inputs.items()}
    x = I["x"].astype(np.float32)
    N = x.shape[0]
    NSH = N // NCORES
    T = (NSH + NPB - 1) // NPB
    NPAD = ((N + P - 1) // P) * P
    NLOCP = ((T * NPB + P - 1) // P) * P
    CH0, CH1, idxs = _preprocess(I["edge_index"], N, NSH)
    W = _fold_weights(I)

    cfg = dict(N=N, NSH=NSH, T=T, NPAD=NPAD, CH0=CH0, CH1=CH1, ncores=NCORES,
               has_dd=bool(np.any(np.abs(W["dd"]) > 0)),
               has_clsb=bool(np.any(np.abs(W["clsb"]) > 0)),
               has_inb=bool(np.any(I["in_b"])))
    assert not cfg["has_inb"]
    assert not np.any(I["c1_bias"]) and not np.any(I["c2_bias"])
    assert np.allclose(I["n1_g"], 1.0) and not np.any(I["n1_b"])
    assert np.allclose(I["n2_g"], 1.0) and not np.any(I["n2_b"])

    iota = np.tile(np.arange(P, dtype=np.float32)[None, :], (P, 1))
    xtf = np.zeros((P, NPAD), np.float32)
    xtf[:, :N] = x.T

    common = {
        "iota_c": _bf16(iota),
        "iotap_c": np.arange(P, dtype=np.float32).reshape(P, 1),
        "ones_c": _bf16(np.ones((1, NPB), np.float32)),
        "xtf": _bf16(xtf),
        "wn1g": _bf16(W["wn1g"]),
        "wn1l": _bf16(W["wn1l"]),
        "wn2": _bf16(W["wn2"]),
        "w1e_bd": _bf16(W["w1e_bd"]),
        "mboth_bd": _bf16(W["mboth_bd"]),
        "eeb1_rep": W["eeb1_rep"],
        "att1_rep": _bf16(W["att1_rep"]),
        "att2_rep": _bf16(W["att2_rep"]),
        "jkw": _bf16(W["jkw"]),
        "clsw": _bf16(W["clsw"]),
        "clsb": W["clsb"],
    }
    in_maps = []
    for k in range(NCORES):
        n0 = k * NSH
        xtl = np.zeros((P, NLOCP), np.float32)
        xtl[:, :NSH] = x[n0:n0 + NSH].T
        m = dict(common)
        m["xtl"] = _bf16(xtl)
        m["idx_src"] = idxs[k][0]
        m["slot_pc"] = _bf16(idxs[k][2])
        m["slot_row"] = _bf16(idxs[k][3])
        in_maps.append(m)
    return cfg, in_maps


def kernel(**inputs):
    global LAST_EXEC_NS, LAST_SCOPES
    from concourse.bass_utils import run_bass_kernel_spmd

    cfg, in_maps = _prepare(inputs)
    key = tuple(sorted((k, v) for k, v in cfg.items()))
    if key not in _CACHE:
        _CACHE[key] = _build(cfg)
    nc = _CACHE[key]
    trace = bool(os.environ.get("KERNEL_TRACE"))
    kw = {}
    if trace:
        import tempfile
        try:
            import ntff_hook  # noqa: F401
        except Exception:
            pass
        kw = dict(trace=True, tmpdir=tempfile.mkdtemp(prefix="ktrace_"))
    res = run_bass_kernel_spmd(nc, in_maps, core_ids=list(range(NCORES)), **kw)
    LAST_EXEC_NS = res.exec_time_ns
    LAST_SCOPES = res.per_core_scope_times
    NSH = cfg["NSH"]
    out = np.concatenate([res.results[k]["out"] for k in range(NCORES)], axis=0)
    return out.astype(np.float32)


# revision 3
# speedup vs baseline: 1.0265x; 1.0265x over previous
"""Trainium2 Bass kernel v2 for nn_DepthAwareGATv2 (2-layer GATv2, 8 cores).

Design (vs baseline): full Python unroll (no hw loops/drains), dma_gather
row-gathers (994ns trigger amortized over a whole tile), global-index node
tables computed fully per core (no ag1), edge-side a_src computed from
gathered xs, dst-side x/adst fetched by dst-keyed gathers from local
tables, single activation table (exp/ln/relu), LN rstd = exp(-0.5*ln(v)).

Per core: nodes [k*6250,(k+1)*6250), edges sharded by dst owner, sorted by
dst, tiled 124 dst-slots/tile, edge slots chunked 128/partition-dim. Edge
order within a tile: src-row group0 (<GS) then group1 (>=GS), each padded
to a 128 multiple; pad edges use src 0 / dst 0 / slot 127.
"""

import math
import os

import numpy as np

NCORES = 8
LAST_EXEC_NS = None
LAST_SCOPES = None
P = 128
NEG_SLOPE = 0.2
NPB = 124
GS = 32768          # int16 gather group split (table row space)

_CACHE = {}


def _bf16(a):
    import ml_dtypes
    return np.ascontiguousarray(
        np.asarray(a, dtype=np.float32).astype(ml_dtypes.bfloat16))


def _pack_idx16(idx, ch):
    """[n] int array -> dma_gather layout [128, ch*8] i16 (i at [i%16,i//16])."""
    n = idx.shape[0]
    assert n == ch * 128
    stripe = idx.astype(np.int16).reshape(ch * 8, 16).T
    return np.ascontiguousarray(np.tile(stripe, (8, 1)))


def _preprocess(edge_index, n, nsh):
    src_all = edge_index[0].astype(np.int64)
    dst_all = edge_index[1].astype(np.int64)
    t_cnt = (nsh + NPB - 1) // NPB
    percore = []
    c0max = c1max = 0
    for k in range(NCORES):
        n0 = k * nsh
        m = (dst_all // nsh) == k
        src, dst = src_all[m], dst_all[m] - n0
        order = np.argsort(dst, kind="stable")
        src, dst = src[order], dst[order]
        tiles = []
        for t in range(t_cnt):
            lo = np.searchsorted(dst, t * NPB)
            hi = np.searchsorted(dst, (t + 1) * NPB)
            s, d = src[lo:hi], dst[lo:hi]
            g0 = s < GS
            s0, d0 = s[g0], d[g0]
            s1, d1 = s[~g0], d[~g0]
            c0 = (len(s0) + P - 1) // P
            c1 = (len(s1) + P - 1) // P
            c0max, c1max = max(c0max, c0), max(c1max, c1)
            tiles.append((s0, d0, s1, d1))
        percore.append(tiles)
    ch0, ch1 = c0max, c1max
    ch = ch0 + ch1
    out = []
    for k, tiles in enumerate(percore):
        src_i16 = np.zeros((t_cnt, P, ch * 8), np.int16)
        dst_i16 = np.zeros((t_cnt, P, ch * 8), np.int16)
        slot = np.full((t_cnt, ch * P), 127.0, np.float32)
        for t, (s0, d0, s1, d1) in enumerate(tiles):
            sg = np.zeros(ch * P, np.int64)
            dg = np.zeros(ch * P, np.int64)
            sl = np.full(ch * P, 127, np.int64)
            sg[:len(s0)] = s0
            dg[:len(d0)] = d0
            sl[:len(s0)] = d0 - t * NPB
            o1 = ch0 * P
            sg[o1:o1 + len(s1)] = s1 - GS
            dg[o1:o1 + len(s1)] = d1
            sl[o1:o1 + len(s1)] = d1 - t * NPB
            src_i16[t] = _pack_idx16(sg, ch)
            dst_i16[t] = _pack_idx16(dg, ch)
            slot[t] = sl.astype(np.float32)
        # slot in edge-major [p, t*ch+c] layout: edge i=c*128+p
        slot_pc = slot.reshape(t_cnt, ch, P).transpose(2, 0, 1).reshape(
            P, t_cnt * ch)
        out.append((src_i16, dst_i16, np.ascontiguousarray(slot_pc), slot))
    return ch0, ch1, out


def _fold_weights(I):
    H, HID = I["c1_att_src"].shape
    F = I["x"].shape[1]

    def v(lin_w, att):
        return (lin_w.reshape(lin_w.shape[0], H, HID) * att[None]).sum(-1)

    W = {}
    G1 = np.einsum("jhc,hc->jh", I["c1_edge_w"].reshape(H, H, HID),
                   I["c1_att_edge"])
    G2 = np.einsum("jhc,hc->jh", I["c2_edge_w"].reshape(H, H, HID),
                   I["c2_att_edge"])
    mboth = np.concatenate([I["ee_w2"] @ G1, I["ee_w2"] @ G2], axis=1)  # [16,16]
    W["mboth_bd"] = np.kron(np.eye(8, dtype=np.float32), mboth)        # [128,128]
    w1e = I["ee_w1"] * (1.0 / F) ** np.arange(1, 5)[:, None]           # [4,16]
    W["w1e_bd"] = np.kron(np.eye(8, dtype=np.float32), w1e)            # [32,128]
    W["eeb1_rep"] = np.tile(I["ee_b1"], 8).reshape(P, 1).astype(np.float32)
    dd = np.concatenate([I["ee_b2"] @ G1, I["ee_b2"] @ G2])            # [16]
    W["dd"] = dd
    eye = np.eye(F, dtype=np.float32)
    W["wn1g"] = np.concatenate(
        [eye, I["c1_lin_w"], v(I["c1_lin_w"], I["c1_att_src"])], axis=1)  # [128,264]
    W["wn1l"] = np.concatenate(
        [eye, v(I["c1_lin_w"], I["c1_att_dst"]), I["in_w"]], axis=1)   # [128,264]
    W["wn2"] = np.concatenate(
        [I["c2_lin_w"], v(I["c2_lin_w"], I["c2_att_src"]),
         v(I["c2_lin_w"], I["c2_att_dst"])], axis=1)                   # [128,144]
    W["asrc1"] = v(I["c1_lin_w"], I["c1_att_src"])                     # [128,8] unused
    # att_src replicated for on-edge a_src: [128, 8,16] rows identical
    W["att1_rep"] = np.tile(I["c1_att_src"].reshape(1, H * HID), (P, 1))
    W["att2_rep"] = np.tile(I["c2_att_src"].reshape(1, H * HID), (P, 1))
    W["jkw"] = np.stack([I["jk_w"][0:128], I["jk_w"][128:256],
                         I["jk_w"][256:384]], axis=1)                  # [128,3,128]
    W["clsw"] = I["cls_w"]
    clsb = I["cls_b"] + I["jk_b"] @ I["cls_w"]
    W["clsb"] = np.tile(clsb[None, :], (P, 1)).astype(np.float32)
    return W


def _build(cfg):
    import concourse.bass as bass
    import concourse.mybir as mybir
    from concourse.bacc import Bacc
    from concourse.tile import TileContext
    from concourse import library_config
    from concourse.masks import make_identity

    f32 = mybir.dt.float32
    bf = mybir.dt.bfloat16
    i16 = mybir.dt.int16
    AF = mybir.ActivationFunctionType
    OP = mybir.AluOpType
    AX = mybir.AxisListType

    NSH = cfg["NSH"]              # 6250
    T = cfg["T"]                  # 51
    CH0, CH1 = cfg["CH0"], cfg["CH1"]
    CH = CH0 + CH1
    NCHK = (NSH + P - 1) // P     # 49 (last chunk 106)
    NPAD = cfg["NPAD"]            # 50048
    JCH = NPAD // P               # 391
    NLOC = T * NPB                # 6324
    NLOCP = ((NLOC + P - 1) // P) * P  # 6400
    ncores = cfg["ncores"]

    nc = Bacc(num_devices=ncores, dynamic_dma_scratch_size=16384,
              num_swdge_queues=4)

    xtf = nc.dram_tensor("xtf", [P, NPAD], bf, kind="ExternalInput")
    xtl = nc.dram_tensor("xtl", [P, NLOCP], bf, kind="ExternalInput")
    idx_src = nc.dram_tensor("idx_src", [T, P, CH * 8], i16, kind="ExternalInput")
    slot_pc = nc.dram_tensor("slot_pc", [P, T * CH], bf, kind="ExternalInput")
    slot_row = nc.dram_tensor("slot_row", [T, CH * P], bf, kind="ExternalInput")
    iota_c = nc.dram_tensor("iota_c", [P, P], bf, kind="ExternalInput")
    iotap_c = nc.dram_tensor("iotap_c", [P, 1], f32, kind="ExternalInput")
    ones_c = nc.dram_tensor("ones_c", [1, NPB], bf, kind="ExternalInput")
    wn1g = nc.dram_tensor("wn1g", [P, 264], bf, kind="ExternalInput")
    wn1l = nc.dram_tensor("wn1l", [P, 264], bf, kind="ExternalInput")
    wn2 = nc.dram_tensor("wn2", [P, 144], bf, kind="ExternalInput")
    w1e_bd = nc.dram_tensor("w1e_bd", [32, P], bf, kind="ExternalInput")
    mboth_bd = nc.dram_tensor("mboth_bd", [P, P], bf, kind="ExternalInput")
    eeb1_rep = nc.dram_tensor("eeb1_rep", [P, 1], f32, kind="ExternalInput")
    att1_rep = nc.dram_tensor("att1_rep", [P, P], bf, kind="ExternalInput")
    att2_rep = nc.dram_tensor("att2_rep", [P, P], bf, kind="ExternalInput")
    jkw = nc.dram_tensor("jkw", [P, 3, P], bf, kind="ExternalInput")
    clsw = nc.dram_tensor("clsw", [P, 40], bf, kind="ExternalInput")
    clsb = nc.dram_tensor("clsb", [P, 40], f32, kind="ExternalInput")

    out_t = nc.dram_tensor("out", [NSH, 40], f32, kind="ExternalOutput")

    with TileContext(nc) as tc:
        with (
            tc.tile_pool(name="dram", bufs=1, space="DRAM") as dpool,
            tc.tile_pool(name="consts", bufs=1) as cpool,
            tc.tile_pool(name="persist", bufs=1) as ppool,
            tc.tile_pool(name="p1pool", bufs=4) as p1pool,
            tc.tile_pool(name="gpool", bufs=2) as gpool,
            tc.tile_pool(name="epool", bufs=2) as epool,
            tc.tile_pool(name="npool", bufs=2) as npool,
            tc.tile_pool(name="pshare", bufs=6, space="PSUM") as pshare,
            tc.tile_pool(name="pagg", bufs=2, space="PSUM") as pagg,
        ):
            shared = {"addr_space": "Shared"} if ncores > 1 else {}
            t1ex = dpool.tile([NPAD, 384], bf, name="t1ex")
            t1loc = dpool.tile([NLOCP, 136], bf, name="t1loc")
            t2loc = dpool.tile([NSH, 128], bf, name="t2loc")
            t2full = dpool.tile([NSH * ncores, 128], bf, name="t2full", **shared)
            dst2ex = dpool.tile([NLOCP, 8], bf, name="dst2ex")
            agg1 = dpool.tile([NLOCP, 136], f32, name="agg1")
            agg2 = dpool.tile([NLOCP, 136], f32, name="agg2")

            def ld(shape, dt_, src, pool=cpool):
                t = pool.tile(shape, dt_, name=f"c_{src.name}")
                nc.sync.dma_start(out=t[:], in_=src[:])
                return t

            iota_t = ld([P, P], bf, iota_c)
            iotap_t = ld([P, 1], f32, iotap_c)
            ones_t = ld([1, NPB], bf, ones_c)
            slot_t = ld([P, T * CH], bf, slot_pc)
            wn1g_t = ld([P, 264], bf, wn1g)
            wn1l_t = ld([P, 264], bf, wn1l)
            wn2_t = ld([P, 144], bf, wn2)
            w1e_t = ld([32, P], bf, w1e_bd)
            mb_t = ld([P, P], bf, mboth_bd)
            eeb1_t = ld([P, 1], f32, eeb1_rep)
            att2_t = ld([P, P], bf, att2_rep)
            jkw_t = ld([P, 3, P], bf, jkw)
            clsw_t = ld([P, 40], bf, clsw)
            clsb_t = ld([P, 40], f32, clsb)
            h0T = ppool.tile([P, NLOCP], bf, name="h0T")
            h1T = ppool.tile([P, NLOCP], bf, name="h1T")
            ae_sb = ppool.tile([P, T, CH, 8], bf, name="ae_sb")
            ident = cpool.tile([P, P], bf, name="ident")
            make_identity(nc, ident[:])
            eps_t = cpool.tile([P, 1], f32, name="eps_t")
            nc.vector.memset(eps_t[:], 1e-5)
            nc.gpsimd.load_library(library_config.mlp)

            # ============ p1: full t1ex table + local t1loc/h0 ============
            with nc.named_scope("p1"):
                zrow = cpool.tile([P, 136], bf, name="zrow")
                nc.vector.memset(zrow[:], 0.0)
                for r0 in range(NSH, NLOCP, P):
                    rw = min(P, NLOCP - r0)
                    nc.sync.dma_start(out=t1loc[bass.ds(r0, rw), :],
                                      in_=zrow[0:rw, :])
                    nc.sync.dma_start(out=dst2ex[bass.ds(r0, rw), :],
                                      in_=zrow[0:rw, 0:8])
                for j in range(JCH):
                    xc = p1pool.tile([P, P], bf, tag="xc")
                    nc.sync.dma_start(out=xc[:], in_=xtf[:, bass.ds(j * P, P)])
                    ps = pshare.tile([P, 264], f32, tag="ps")
                    nc.tensor.matmul(out=ps[:, 0:264], lhsT=xc[:], rhs=wn1g_t[:],
                                     start=True, stop=True)
                    row = p1pool.tile([P, 264], bf, tag="p1row")
                    nc.scalar.activation(out=row[:], in_=ps[:, 0:264],
                                         func=AF.Copy)
                    nc.sync.dma_start(out=t1ex[bass.ds(j * P, P), 0:264],
                                      in_=row[:])
                for j in range(NCHK):
                    nr = min(P, NSH - j * P)
                    xc = p1pool.tile([P, P], bf, tag="xlc")
                    nc.sync.dma_start(out=xc[:, 0:nr],
                                      in_=xtl[:, bass.ds(j * P, nr)])
                    ps = pshare.tile([P, 264], f32, tag="ps")
                    nc.tensor.matmul(out=ps[0:nr, :], lhsT=xc[:, 0:nr],
                                     rhs=wn1l_t[:], start=True, stop=True)
                    row = p1pool.tile([P, 136], bf, tag="p1lrow")
                    nc.vector.tensor_copy(out=row[0:nr, :], in_=ps[0:nr, 0:136])
                    nc.sync.dma_start(out=t1loc[bass.ds(j * P, nr), 0:136],
                                      in_=row[0:nr, :])
                    h0b = p1pool.tile([P, P], bf, tag="h0b")
                    nc.vector.tensor_copy(out=h0b[0:nr, :], in_=ps[0:nr, 136:264])
                    h0tp = pshare.tile([P, P], bf, tag="ps")
                    nc.tensor.transpose(out=h0tp[:, 0:nr], in_=h0b[0:nr, :],
                                        identity=ident[0:nr, 0:nr])
                    nc.vector.tensor_copy(out=h0T[:, bass.ds(j * P, nr)],
                                          in_=h0tp[:, 0:nr])

            # ============ edge passes ============
            qn = [0]

            def layer_pass(scope, layer, tblg, tbld, agg_dst):
                xs_off = 128 if layer == 1 else 0
                as_off = 256
                gcols = 384 if layer == 1 else 128
                dcols = 136 if layer == 1 else 8
                with nc.named_scope(scope):
                    for t in range(T):
                        isrc = gpool.tile([P, CH * 8], i16, tag="isrc", bufs=3)
                        nc.sync.dma_start(out=isrc[:], in_=idx_src[t])
                        g = gpool.tile([P, CH, gcols], bf, tag="g", bufs=2)
                        for g0, g1, base in ((0, CH0, 0), (CH0, CH, GS)):
                            c0 = g0
                            while c0 < g1:
                                w = min(7, g1 - c0)
                                src_ap = (tblg[:] if base == 0 else
                                          tblg[bass.ds(GS, tblg.shape[0] - GS), :])
                                nc.gpsimd.dma_gather(
                                    g[:, c0:c0 + w, :], src_ap,
                                    isrc[:, c0 * 8:(c0 + w) * 8],
                                    w * P, w * P, gcols,
                                    queue_num=qn[0] % 4)
                                qn[0] += 1
                                c0 += w
                        # dst-side: 124 rows -> ohT -> per-chunk broadcast
                        dslots = gpool.tile([NPB, dcols], bf, tag="dsl")
                        nc.sync.dma_start(out=dslots[:],
                                          in_=tbld[bass.ds(t * NPB, NPB), :])
                        srow = gpool.tile([1, CH * P], bf, tag="srow",
                                          bufs=2)
                        nc.sync.dma_start(out=srow[:],
                                          in_=slot_row[bass.ds(t, 1), :])
                        ohT = gpool.tile([NPB, CH * P], bf, tag="ohT")
                        for q0 in range(0, CH * P, 512):
                            qw = min(512, CH * P - q0)
                            sb = pshare.tile([NPB, 512], f32, tag="ps")
                            nc.tensor.matmul(
                                out=sb[:, 0:qw], lhsT=ones_t[:],
                                rhs=srow[:, bass.ds(q0, qw)],
                                start=True, stop=True)
                            nc.vector.tensor_scalar(
                                out=ohT[:, q0:q0 + qw], in0=sb[:, 0:qw],
                                scalar1=iotap_t[0:NPB, :], scalar2=None,
                                op0=OP.is_equal)
                        xda = gpool.tile([P, CH, dcols], bf, tag="xda")
                        for c in range(CH):
                            xd_ps = pshare.tile([P, dcols], f32, tag="ps")
                            nc.tensor.matmul(
                                out=xd_ps[:], lhsT=ohT[:, bass.ds(c * P, P)],
                                rhs=dslots[:], start=True, stop=True)
                            nc.scalar.activation(out=xda[:, c, :],
                                                 in_=xd_ps[:], func=AF.Copy)
                        slot_v = slot_t[:, bass.ds(t * CH, CH)]
                        # z = a_src + a_dst(bcast) + ae
                        z = gpool.tile([P, CH, 8], f32, tag="z")
                        if layer == 1:
                            nc.vector.tensor_tensor(
                                out=z[:], in0=g[:, :, as_off:as_off + 8],
                                in1=xda[:, :, 128:136], op=OP.add)
                        else:
                            zs4 = gpool.tile([P, CH, 8, 16], bf, tag="zs4")
                            nc.vector.tensor_tensor(
                                out=zs4[:],
                                in0=g[:, :, 0:128].rearrange(
                                    "p c (h q) -> p c h q", h=8),
                                in1=att2_t[:, None, :].rearrange(
                                    "p o (h q) -> p o h q", h=8).to_broadcast(
                                        [P, CH, 8, 16]),
                                op=OP.mult)
                            nc.vector.tensor_reduce(out=z[:], in_=zs4[:],
                                                    op=OP.add, axis=AX.X)
                            nc.vector.tensor_tensor(out=z[:], in0=z[:],
                                                    in1=xda[:], op=OP.add)
                        if layer == 1:
                            # s-dot + edge MLP -> ae (both layers)
                            prod = gpool.tile([P, CH, P], bf, tag="prod")
                            nc.vector.tensor_tensor(out=prod[:],
                                                    in0=g[:, :, 0:128],
                                                    in1=xda[:, :, 0:128],
                                                    op=OP.mult)
                            s = gpool.tile([P, CH], f32, tag="s")
                            nc.vector.tensor_reduce(out=s[:], in_=prod[:],
                                                    op=OP.add, axis=AX.X)
                            s2 = gpool.tile([P, CH], f32, tag="s2")
                            nc.vector.tensor_tensor(out=s2[:], in0=s[:],
                                                    in1=s[:], op=OP.mult)
                            p4 = gpool.tile([P, CH, 4], bf, tag="p4")
                            nc.vector.tensor_copy(out=p4[:, :, 0], in_=s[:])
                            nc.vector.tensor_copy(out=p4[:, :, 1], in_=s2[:])
                            s34 = gpool.tile([P, CH], f32, tag="s34")
                            nc.vector.tensor_tensor(out=s34[:], in0=s2[:],
                                                    in1=s[:], op=OP.mult)
                            nc.vector.tensor_copy(out=p4[:, :, 2], in_=s34[:])
                            nc.vector.tensor_tensor(out=s34[:], in0=s2[:],
                                                    in1=s2[:], op=OP.mult)
                            nc.vector.tensor_copy(out=p4[:, :, 3], in_=s34[:])
                            ae_t = gpool.tile([P, CH, 16], bf, tag="ae_t")
                            for gi in range(0, CH, 8):
                                w = min(8, CH - gi)
                                ptp = pshare.tile([32, P], bf, tag="ps")
                                nc.tensor.transpose(
                                    out=ptp[0:4 * w, :],
                                    in_=p4[:, gi:gi + w, :].rearrange(
                                        "p c f -> p (c f)"),
                                    identity=ident[:])
                                p4T = gpool.tile([32, P], bf, tag="p4T")
                                nc.scalar.activation(out=p4T[0:4 * w, :],
                                                     in_=ptp[0:4 * w, :],
                                                     func=AF.Copy)
                                hid = pshare.tile([P, P], f32, tag="ps")
                                nc.tensor.matmul(out=hid[0:16 * w, :],
                                                 lhsT=w1e_t[0:4 * w, 0:16 * w],
                                                 rhs=p4T[0:4 * w, :],
                                                 start=True, stop=True)
                                hidr = gpool.tile([P, P], bf, tag="hidr")
                                nc.scalar.activation(
                                    out=hidr[0:16 * w, :], in_=hid[0:16 * w, :],
                                    func=AF.Relu, bias=eeb1_t[0:16 * w, :])
                                aeps = pshare.tile([P, P], f32, tag="ps")
                                nc.tensor.matmul(out=aeps[:, 0:16 * w],
                                                 lhsT=hidr[0:16 * w, :],
                                                 rhs=mb_t[0:16 * w, 0:16 * w],
                                                 start=True, stop=True)
                                nc.scalar.activation(
                                    out=ae_t[:, gi:gi + w, :],
                                    in_=aeps[:, 0:16 * w].rearrange(
                                        "p (c f) -> p c f", f=16),
                                    func=AF.Copy)
                            nc.vector.tensor_copy(out=ae_sb[:, t, :, :],
                                                  in_=ae_t[:, :, 8:16])
                            ae_l = ae_t[:, :, 0:8]
                        else:
                            ae_l = ae_sb[:, t, :, :]
                        nc.vector.tensor_tensor(out=z[:], in0=z[:], in1=ae_l,
                                                op=OP.add)
                        if cfg["has_dd"]:
                            raise NotImplementedError
                        zz = gpool.tile([P, CH, 8], f32, tag="zz")
                        nc.vector.tensor_scalar(out=zz[:], in0=z[:],
                                                scalar1=NEG_SLOPE, scalar2=None,
                                                op0=OP.mult)
                        nc.vector.tensor_tensor(out=z[:], in0=z[:], in1=zz[:],
                                                op=OP.max)
                        mez = gpool.tile([P, CH, 136], bf, tag="mez")
                        nc.scalar.activation(out=mez[:, :, 128:136], in_=z[:],
                                             func=AF.Exp)
                        nc.vector.tensor_tensor(
                            out=mez[:, :, 0:128].rearrange(
                                "p c (h q) -> p c h q", h=8),
                            in0=g[:, :, xs_off:xs_off + 128].rearrange(
                                "p c (h q) -> p c h q", h=8),
                            in1=mez[:, :, 128:136, None].to_broadcast(
                                [P, CH, 8, 16]),
                            op=OP.mult)
                        oh = gpool.tile([P, CH, NPB], bf, tag="oh")
                        nc.vector.tensor_tensor(
                            out=oh[:],
                            in0=iota_t[:, None, 0:NPB].to_broadcast(
                                [P, CH, NPB]),
                            in1=slot_v[:, :, None].to_broadcast([P, CH, NPB]),
                            op=OP.is_equal)
                        aggp = pagg.tile([NPB, 136], f32, tag="aggp")
                        for c in range(CH):
                            nc.tensor.matmul(out=aggp[:], lhsT=oh[:, c, :],
                                             rhs=mez[:, c, :],
                                             start=(c == 0), stop=(c == CH - 1))
                        aggs = gpool.tile([NPB, 136], f32, tag="aggs")
                        nc.vector.tensor_copy(out=aggs[:], in_=aggp[:])
                        nc.sync.dma_start(out=agg_dst[bass.ds(t * NPB, NPB), :],
                                          in_=aggs[:])

            layer_pass("passA", 1, t1ex, t1loc, agg1)

            # ============ epilogues ============
            def epilogue(scope, agg_src, hprevT, build_t2):
                with nc.named_scope(scope):
                    for j in range(NCHK):
                        nr = min(P, NSH - j * P)
                        ag = epool.tile([P, 136], f32, tag="ag")
                        nc.sync.dma_start(out=ag[0:nr, :],
                                          in_=agg_src[bass.ds(j * P, nr), :])
                        rden = epool.tile([P, 8], f32, tag="rden")
                        nc.vector.reciprocal(out=rden[0:nr, :],
                                             in_=ag[0:nr, 128:136])
                        o1 = epool.tile([P, P], f32, tag="o1")
                        nc.vector.tensor_tensor(
                            out=o1[0:nr, :].rearrange("p (h q) -> p h q", h=8),
                            in0=ag[0:nr, 0:128].rearrange(
                                "p (h q) -> p h q", h=8),
                            in1=rden[0:nr, :, None].to_broadcast([nr, 8, 16]),
                            op=OP.mult)
                        # elu + residual(hprev)
                        mn = epool.tile([P, P], f32, tag="mn")
                        nc.vector.tensor_scalar(out=mn[0:nr, :], in0=o1[0:nr, :],
                                                scalar1=0.0, scalar2=None,
                                                op0=OP.min)
                        ex = epool.tile([P, P], f32, tag="ex")
                        nc.scalar.activation(out=ex[0:nr, :], in_=mn[0:nr, :],
                                             func=AF.Exp)
                        h = epool.tile([P, P], f32, tag="h")
                        nc.vector.tensor_scalar(out=h[0:nr, :], in0=o1[0:nr, :],
                                                scalar1=0.0, scalar2=None,
                                                op0=OP.max)
                        nc.vector.tensor_tensor(out=h[0:nr, :], in0=h[0:nr, :],
                                                in1=ex[0:nr, :], op=OP.add)
                        hptp = pshare.tile([P, P], bf, tag="ps")
                        nc.tensor.transpose(out=hptp[0:nr, :],
                                            in_=hprevT[:, bass.ds(j * P, nr)],
                                            identity=ident[:])
                        nc.vector.tensor_tensor(out=h[0:nr, :], in0=h[0:nr, :],
                                                in1=hptp[0:nr, :], op=OP.add)
                        nc.vector.tensor_scalar(out=h[0:nr, :], in0=h[0:nr, :],
                                                scalar1=-1.0, scalar2=None,
                                                op0=OP.add)
                        # layernorm: rstd = exp(-0.5*ln(var+eps))
                        msum = epool.tile([P, 1], f32, tag="msum")
                        nc.vector.tensor_reduce(out=msum[0:nr, :],
                                                in_=h[0:nr, :], op=OP.add,
                                                axis=AX.X)
                        nc.vector.tensor_scalar(out=msum[0:nr, :],
                                                in0=msum[0:nr, :],
                                                scalar1=1.0 / 128, scalar2=None,
                                                op0=OP.mult)
                        xc = epool.tile([P, P], f32, tag="xc")
                        nc.vector.tensor_scalar(out=xc[0:nr, :], in0=h[0:nr, :],
                                                scalar1=msum[0:nr, :],
                                                scalar2=None, op0=OP.subtract)
                        sq = epool.tile([P, P], f32, tag="sq")
                        nc.vector.tensor_tensor(out=sq[0:nr, :], in0=xc[0:nr, :],
                                                in1=xc[0:nr, :], op=OP.mult)
                        vs = epool.tile([P, 1], f32, tag="vs")
                        nc.vector.tensor_reduce(out=vs[0:nr, :], in_=sq[0:nr, :],
                                                op=OP.add, axis=AX.X)
                        lnv = epool.tile([P, 1], f32, tag="lnv")
                        nc.scalar.activation(out=lnv[0:nr, :], in_=vs[0:nr, :],
                                             func=AF.Ln, scale=1.0 / 128,
                                             bias=eps_t[0:nr, :])
                        rstd = epool.tile([P, 1], f32, tag="rstd")
                        nc.scalar.activation(out=rstd[0:nr, :], in_=lnv[0:nr, :],
                                             func=AF.Exp, scale=-0.5)
                        hln = epool.tile([P, P], f32, tag="hln")
                        nc.vector.tensor_scalar(out=hln[0:nr, :],
                                                in0=xc[0:nr, :],
                                                scalar1=rstd[0:nr, :],
                                                scalar2=None, op0=OP.mult)
                        hb = epool.tile([P, P], bf, tag="hb")
                        nc.vector.tensor_copy(out=hb[0:nr, :], in_=hln[0:nr, :])
                        htp = pshare.tile([P, P], bf, tag="ps")
                        nc.tensor.transpose(out=htp[:, 0:nr], in_=hb[0:nr, :],
                                            identity=ident[0:nr, 0:nr])
                        if build_t2:
                            nc.vector.tensor_copy(
                                out=h1T[:, bass.ds(j * P, nr)],
                                in_=htp[:, 0:nr])
                            t2ps = pshare.tile([P, 136], f32, tag="ps")
                            nc.tensor.matmul(out=t2ps[0:nr, :],
                                             lhsT=h1T[:, bass.ds(j * P, nr)],
                                             rhs=wn2_t[:], start=True, stop=True)
                            t2row = epool.tile([P, P], bf, tag="t2row")
                            nc.vector.tensor_copy(out=t2row[0:nr, :],
                                                  in_=t2ps[0:nr, 0:128])
                            nc.sync.dma_start(out=t2loc[bass.ds(j * P, nr), :],
                                              in_=t2row[0:nr, :])
                            adrow = epool.tile([P, 8], bf, tag="adrow")
                            nc.vector.tensor_copy(out=adrow[0:nr, :],
                                                  in_=t2ps[0:nr, 128:136])
                            nc.sync.dma_start(
                                out=dst2ex[bass.ds(j * P, nr), :],
                                in_=adrow[0:nr, :])
                        else:
                            h2T = npool.tile([P, P], bf, tag="h2T")
                            nc.vector.tensor_copy(out=h2T[:, 0:nr],
                                                  in_=htp[:, 0:nr])
                            hh = pshare.tile([P, P], f32, tag="ps")
                            nc.tensor.matmul(out=hh[:, 0:nr], lhsT=jkw_t[:, 0, :],
                                             rhs=h0T[:, bass.ds(j * P, nr)],
                                             start=True, stop=False)
                            nc.tensor.matmul(out=hh[:, 0:nr], lhsT=jkw_t[:, 1, :],
                                             rhs=h1T[:, bass.ds(j * P, nr)],
                                             start=False, stop=False)
                            nc.tensor.matmul(out=hh[:, 0:nr], lhsT=jkw_t[:, 2, :],
                                             rhs=h2T[:, 0:nr],
                                             start=False, stop=True)
                            hhb = npool.tile([P, P], bf, tag="hhb")
                            nc.vector.tensor_copy(out=hhb[:, 0:nr],
                                                  in_=hh[:, 0:nr])
                            lg = pshare.tile([P, 40], f32, tag="ps")
                            nc.tensor.matmul(out=lg[0:nr, :],
                                             lhsT=hhb[:, 0:nr], rhs=clsw_t[:],
                                             start=True, stop=True)
                            if cfg["has_clsb"]:
                                raise NotImplementedError
                            rmax = npool.tile([P, 1], f32, tag="rmax")
                            nc.vector.tensor_reduce(out=rmax[0:nr, :],
                                                    in_=lg[0:nr, :], op=OP.max,
                                                    axis=AX.X)
                            xm = npool.tile([P, 40], f32, tag="xm")
                            nc.vector.tensor_scalar(out=xm[0:nr, :],
                                                    in0=lg[0:nr, :],
                                                    scalar1=rmax[0:nr, :],
                                                    scalar2=None,
                                                    op0=OP.subtract)
                            ee = npool.tile([P, 40], f32, tag="ee")
                            esum = npool.tile([P, 1], f32, tag="esum")
                            nc.scalar.activation(out=ee[0:nr, :], in_=xm[0:nr, :],
                                                 func=AF.Exp,
                                                 accum_out=esum[0:nr, :])
                            lse = npool.tile([P, 1], f32, tag="lse")
                            nc.scalar.activation(out=lse[0:nr, :],
                                                 in_=esum[0:nr, :], func=AF.Ln)
                            res = npool.tile([P, 40], f32, tag="res")
                            nc.vector.tensor_scalar(out=res[0:nr, :],
                                                    in0=xm[0:nr, :],
                                                    scalar1=lse[0:nr, :],
                                                    scalar2=None,
                                                    op0=OP.subtract)
                            nc.sync.dma_start(out=out_t[bass.ds(j * P, nr), :],
                                              in_=res[0:nr, :])

            epilogue("ep1", agg1, h0T, True)

            with nc.named_scope("ag2"):
                if ncores > 1:
                    import concourse.mybir as mybir2
                    nc.gpsimd.collective_compute(
                        "AllGather", mybir2.AluOpType.bypass,
                        replica_groups=[list(range(ncores))],
                        ins=[t2loc[:]], outs=[t2full[:]],
                    )
                else:
                    nc.sync.dma_start(out=t2full[:], in_=t2loc[:])

            layer_pass("passB", 2, t2full, dst2ex, agg2)
            epilogue("ep2", agg2, h1T, False)

    nc.finalize()
    return nc


def _prepare(inputs):
    I = {k: np.asarray(v) for k, v in inputs.items()}
    x = I["x"].astype(np.float32)
    N = x.shape[0]
    NSH = N // NCORES
    T = (NSH + NPB - 1) // NPB
    NPAD = ((N + P - 1) // P) * P
    NLOCP = ((T * NPB + P - 1) // P) * P
    CH0, CH1, idxs = _preprocess(I["edge_index"], N, NSH)
    W = _fold_weights(I)

    cfg = dict(N=N, NSH=NSH, T=T, NPAD=NPAD, CH0=CH0, CH1=CH1, ncores=NCORES,
               has_dd=bool(np.any(np.abs(W["dd"]) > 0)),
               has_clsb=bool(np.any(np.abs(W["clsb"]) > 0)),
               has_inb=bool(np.any(I["in_b"])))
    assert not cfg["has_inb"]
    assert not np.any(I["c1_bias"]) and not np.any(I["c2_bias"])
    assert np.allclose(I["n1_g"], 1.0) and not np.any(I["n1_b"])
    assert np.allclose(I["n2_g"], 1.0) and not np.any(I["n2_b"])

    iota = np.tile(np.arange(P, dtype=np.float32)[None, :], (P, 1))
    xtf = np.zeros((P, NPAD), np.float32)
    xtf[:, :N] = x.T

    common = {
        "iota_c": _bf16(iota),
        "iotap_c": np.arange(P, dtype=np.float32).reshape(P, 1),
        "ones_c": _bf16(np.ones((1, NPB), np.float32)),
        "xtf": _bf16(xtf),
        "wn1g": _bf16(W["wn1g"]),
        "wn1l": _bf16(W["wn1l"]),
        "wn2": _bf16(W["wn2"]),
        "w1e_bd": _bf16(W["w1e_bd"]),
        "mboth_bd": _bf16(W["mboth_bd"]),
        "eeb1_rep": W["eeb1_rep"],
        "att1_rep": _bf16(W["att1_rep"]),
        "att2_rep": _bf16(W["att2_rep"]),
        "jkw": _bf16(W["jkw"]),
        "clsw": _bf16(W["clsw"]),
        "clsb": W["clsb"],
    }
    in_maps = []
    for k in range(NCORES):
        n0 = k * NSH
        xtl = np.zeros((P, NLOCP), np.float32)
        xtl[:, :NSH] = x[n0:n0 + NSH].T
        m = dict(common)
        m["xtl"] = _bf16(xtl)
        m["idx_src"] = idxs[k][0]
        m["slot_pc"] = _bf16(idxs[k][2])
        m["slot_row"] = _bf16(idxs[k][3])
        in_maps.append(m)
    return cfg, in_maps


def kernel(**inputs):
    global LAST_EXEC_NS, LAST_SCOPES
    from concourse.bass_utils import run_bass_kernel_spmd

    cfg, in_maps = _prepare(inputs)
    key = tuple(sorted((k, v) for k, v in cfg.items()))
    if key not in _CACHE:
        _CACHE[key] = _build(cfg)
    nc = _CACHE[key]
    trace = bool(os.environ.get("KERNEL_TRACE"))
    kw = {}
    if trace:
        import tempfile
        try:
            import ntff_hook  # noqa: F401
        except Exception:
            pass
        kw = dict(trace=True, tmpdir=tempfile.mkdtemp(prefix="ktrace_"))
    res = run_bass_kernel_spmd(nc, in_maps, core_ids=list(range(NCORES)), **kw)
    LAST_EXEC_NS = res.exec_time_ns
    LAST_SCOPES = res.per_core_scope_times
    NSH = cfg["NSH"]
    out = np.concatenate([res.results[k]["out"] for k in range(NCORES)], axis=0)
    return out.astype(np.float32)
